# revision 19
# baseline (speedup 1.0000x reference)
"""Trainium2 Bass kernel for nn_CNNLR (CNN + quadratic-expansion + linear regression).

Math: out[n] = w0 + w1 . f[n] + f[n]^T U f[n], where f[n] (1664 = 26 pos x 64 ch)
are the conv features and U is the block-upper-triangular reshape of the second
order part of the 1.33M-wide reg weight.

Strategy (8 cores, one uniform SPMD program): the convolutions are 0.2% of the
FLOPs and are computed exactly on the host (they are needed there anyway for
the final dot); the device does only the dominant quadratic-partial contraction

    v[n, t'] = sum_{t < t'} f[n, t] U[t, t']   (1664 x 1664 upper-triangular)

sharded by t'-chunks of 128 columns (13 chunks over 8 cores). Each core gets
one DEEP chunk slot (all 13 position-pairs of contraction) and one SHALLOW
slot (chunks 0-5 only need pairs 0-5), which trims the zero-padding the
block-triangular structure forces on an SPMD-uniform program from 2x to ~1.5x.
Features are pair-packed [(c2, par), pair, batch] so each 128-col matmul
contracts 128 deep. f and U ship in bf16 (fp32 psum accumulation; measured
rel err ~1.7e-3 vs the 2e-2 gate). The host applies v . f, the first-order
term and constants in float64.

All input DMAs ride the sync HWDGE ring as 128-partition transfers (the SDMA
fan-out follows SBUF partition groups; this shape hits ~400GB/s across all 16
engines), ordered so the shallow quad can start while the deep U still
streams. Dummy bf16 matmuls bridge the PE HAM clock-gate window during the
DMA wait. Set BASS_KERNEL_DTYPE=fp32 for a full-precision (fp32r) fallback.
"""

import os
import sys

sys.path.insert(0, "/opt/trn_rl_repo")

import numpy as np

B = 128          # batch
L = 26           # positions
C1, C2 = 128, 64
K1, K2 = 7, 5
NPOS = 25
NFEAT = L * C2   # 1664
H = 1 + NFEAT + (C2 * C2) * (NPOS * (NPOS + 1) // 2)

NCORES = 8
NTC = 13         # t' chunks of 128 (= 2 positions each)
NPAIR = 13       # position pairs (26 positions / 2)
LP = L + 4       # conv2 halo used by the host conv
NSH = 6          # pair depth of the shallow slot (chunks 0-5 need pairs 0-5)

# per-core chunk assignment: deep slot (pairs 0-12) and shallow slot (pairs 0-5)
DEEP = [6, 7, 8, 9, 10, 11, 12, -1]
SHAL = [0, 1, 2, 3, 4, 5, -1, -1]

DTYPE = os.environ.get("BASS_KERNEL_DTYPE", "bf16")  # "bf16" | "fp32"

_CACHE: dict = {}


def _np_qdt():
    import ml_dtypes

    return np.dtype(ml_dtypes.bfloat16) if DTYPE == "bf16" else np.dtype(np.float32)


def _build_program():
    import concourse.mybir as mybir
    import concourse.tile as tile
    from concourse import bacc

    f32 = mybir.dt.float32
    qdt = mybir.dt.bfloat16 if DTYPE == "bf16" else mybir.dt.float32r
    wdt = qdt if DTYPE == "bf16" else f32  # warm tile (f32r memset is rejected)
    nc = bacc.Bacc(
        "TRN2",
        target_bir_lowering=False,
        debug=False,
        enable_asserts=False,
        num_devices=NCORES,
    )

    FT = nc.dram_tensor("ft_p", [128, NPAIR, B], qdt, kind="ExternalInput").ap()
    # merged U: j < NSH -> [deep_j | shallow_j] 256-col blocks; j >= NSH -> 128
    UQ = nc.dram_tensor("uq_m", [128, NSH * 256 + (NPAIR - NSH) * 128], qdt,
                        kind="ExternalInput").ap()
    VDP = nc.dram_tensor("v_dp", [B, 128], qdt, kind="ExternalOutput").ap()
    VSH = nc.dram_tensor("v_sh", [B, 128], qdt, kind="ExternalOutput").ap()

    with tile.TileContext(nc) as tc:
        with (
            tc.tile_pool(name="const", bufs=1) as cpool,
            tc.tile_pool(name="work", bufs=1) as wpool,
            tc.tile_pool(name="psw", bufs=2, space="PSUM") as psw,
            tc.tile_pool(name="psv", bufs=1, space="PSUM") as psv,
        ):
            ft2 = cpool.tile([128, NPAIR, B], qdt)
            uqm = cpool.tile([128, NSH * 256 + (NPAIR - NSH) * 128], qdt)
            warm = wpool.tile([C1, 256], wdt)
            vodp = wpool.tile([B, 128], qdt)
            vosh = wpool.tile([B, 128], qdt)

            # input DMAs first, all on the sync ring, interleaved in
            # consumption order so matmul j can start as soon as its ft2
            # slice and U block have landed
            nc.gpsimd.memset(warm[:], 0.0)
            nc.sync.dma_start(ft2[:, 0:7, :], FT[:, 0:7, :])
            nc.sync.dma_start(uqm[:, 0:1536], UQ[:, 0:1536])
            nc.sync.dma_start(ft2[:, 7:NPAIR, :], FT[:, 7:NPAIR, :])
            nc.sync.dma_start(uqm[:, 1536:2432], UQ[:, 1536:2432])

            # HAM warmup: dummy matmuls keep the PE alive while inputs
            # stream in. All accumulate into one psum tile (never read) so
            # there is no buffer rotation to serialize them.
            NWARM = 16
            wps = psw.tile([C1, 256], f32, tag="wps")
            for k in range(NWARM):
                nc.tensor.matmul(
                    wps[:], warm[:, :128], warm[:],
                    start=(k == 0), stop=(k == NWARM - 1),
                    skip_group_check=True,
                )

            # v[n, :]: cols 0:128 = deep chunk (pairs 0-12), cols 128:256 =
            # shallow chunk (pairs 0-5). Pairs j < NSH feed both chunks with
            # ONE 256-col matmul (shared ft2 weight load); j >= NSH are
            # 128-col deep-only. has_written bits make the two column
            # ranges independent accumulation streams within one psum bank.
            vps = psv.tile([B, 256], f32)
            for j in range(NPAIR):
                if j < NSH:
                    rhs = uqm[:, j * 256 : (j + 1) * 256]
                    dst = vps[:]
                else:
                    rhs = uqm[:, NSH * 256 + (j - NSH) * 128 :
                              NSH * 256 + (j - NSH + 1) * 128]
                    dst = vps[:, 0:128]
                nc.tensor.matmul(
                    dst,
                    ft2[:, j, :],
                    rhs,
                    start=(j == 0),
                    stop=(j == NPAIR - 1),
                    skip_group_check=True,
                )
            nc.vector.tensor_copy(vodp[:], vps[:, 0:128])
            nc.vector.tensor_copy(vosh[:], vps[:, 128:256])
            nc.sync.dma_start(VDP[:], vodp[:])
            nc.sync.dma_start(VSH[:], vosh[:])

    nc.compile()
    return nc


def _get_program():
    if "nc" not in _CACHE:
        _CACHE["nc"] = _build_program()
    return _CACHE["nc"]


def _host_conv1(x, conv1_w, conv1_b):
    """Exact conv1 + ReLU on host via embedding gather (input is one-hot).

    Returns h1 in layout [C1, LP, B] with zero halo columns."""
    xpad = np.full((B, L + K1 - 1), 4, np.int64)  # 4 = pad token
    xpad[:, K1 // 2 : K1 // 2 + L] = np.asarray(x).astype(np.int64)
    w1g = np.zeros((K1, 5, C1), np.float32)
    w1g[:, :4, :] = np.asarray(conv1_w, np.float32).transpose(2, 1, 0)
    y1 = np.zeros((B, L, C1), np.float32)
    for t in range(K1):
        y1 += w1g[t][xpad[:, t : t + L]]
    h1nlc = np.maximum(y1 + np.asarray(conv1_b, np.float32)[None, None, :], 0.0)
    h1 = np.zeros((C1, LP, B), np.float32)
    h1[:, 2 : 2 + L, :] = h1nlc.transpose(2, 1, 0)
    return h1


def _host_feat(h1, w2, b2):
    """Exact fp32 conv2 features on host, [B, NFEAT] position-major."""
    y2 = np.zeros((C2, L, B), np.float32)
    for t in range(K2):
        y2 += np.einsum(
            "cd,cln->dln", w2[:, t * C2 : (t + 1) * C2], h1[:, t : t + L, :]
        )
    ft = np.maximum(y2 + b2[:, :, None], 0.0)
    return ft.transpose(2, 1, 0).reshape(B, NFEAT)


def _host_prep(x, conv1_w, conv1_b, conv2_w, conv2_b, reg_w):
    """Build per-core input maps (layouts match the program)."""
    conv1_w = np.asarray(conv1_w, np.float32)
    conv1_b = np.asarray(conv1_b, np.float32)
    conv2_w = np.asarray(conv2_w, np.float32)
    conv2_b = np.asarray(conv2_b, np.float32)
    reg_w = np.asarray(reg_w, np.float32)

    # exact features (also used for the host-side dot / first-order term)
    h1 = _host_conv1(x, conv1_w, conv1_b)                  # [C1, LP, B]
    w2 = conv2_w.transpose(1, 2, 0).reshape(C1, K2 * C2)   # [c1, t*C2+c2]
    b2 = np.ascontiguousarray(conv2_b.reshape(C2, 1))
    feat = _host_feat(h1, w2, b2)

    # pair-packed device features: ft2[(c2 + 64*par), j, n] = f[n, (2j+par)*64+c2]
    ft2 = np.ascontiguousarray(
        feat.reshape(B, NPAIR, 2, C2).transpose(2, 3, 1, 0).reshape(128, NPAIR, B)
    )

    # second-order weight blocks: blocks[i][j, p-(i+1), k] = U[i*64+j, p*64+k]
    w2nd = reg_w[0, 1 + NFEAT :]
    sizes = [(NPOS - i) * C2 * C2 for i in range(NPOS)]
    offs = np.concatenate([[0], np.cumsum(sizes)])
    blocks = [
        w2nd[offs[i] : offs[i + 1]].reshape(C2, NPOS - i, C2) for i in range(NPOS)
    ]

    def pack_chunk(a, npair):
        """U columns for t'-chunk a (positions 2a, 2a+1), pair-packed rows."""
        u = np.zeros((128, npair, 128), np.float32)
        if a < 0:
            return u
        for p in (2 * a, 2 * a + 1):
            if p < 1 or p > NPOS:
                continue
            c0 = (p - 2 * a) * C2
            for i in range(p):
                u[64 * (i % 2) : 64 * (i % 2) + C2, i // 2, c0 : c0 + C2] = (
                    blocks[i][:, p - i - 1, :]
                )
        return u

    qnp = _np_qdt()
    ftq = ft2.astype(qnp)
    in_maps = []
    for core in range(NCORES):
        ua = pack_chunk(DEEP[core], NPAIR)
        ub = pack_chunk(SHAL[core], NSH)
        uqm = np.zeros((128, NSH * 256 + (NPAIR - NSH) * 128), np.float32)
        for j in range(NSH):
            uqm[:, j * 256 : j * 256 + 128] = ua[:, j, :]
            uqm[:, j * 256 + 128 : (j + 1) * 256] = ub[:, j, :]
        for j in range(NSH, NPAIR):
            uqm[:, NSH * 256 + (j - NSH) * 128 : NSH * 256 + (j - NSH + 1) * 128] = (
                ua[:, j, :]
            )
        in_maps.append({"ft_p": ftq, "uq_m": uqm.astype(qnp)})
    return in_maps, feat


def _host_post(results, feat, reg_w, reg_b):
    reg_w = np.asarray(reg_w, np.float32)
    reg_b = np.asarray(reg_b, np.float32)
    feat = feat.astype(np.float64)

    w1vec = reg_w[0, 1 : 1 + NFEAT].astype(np.float64)
    out = feat @ w1vec + np.float64(reg_w[0, 0]) + np.float64(reg_b[0])

    feat2 = feat.reshape(B, NTC, 128)
    for core in range(NCORES):
        for key, a in (("v_dp", DEEP[core]), ("v_sh", SHAL[core])):
            if a < 0:
                continue
            vt = results[core][key].astype(np.float64)  # [B, 128]
            out += np.einsum("nr,nr->n", vt, feat2[:, a, :])
    return out.astype(np.float32)


def _install_ntff_shim():
    """Register the axon NTFF profile hook that the agent image's antenv lacks.

    Replicates trn_boot._ntff_profile_via_ctypes against /opt/axon/libaxon_pjrt.so
    and exposes it via a synthetic antenv.axon_hooks module so that
    bass_utils.run_bass_kernel_spmd(trace=True) can find it.
    """
    import sys as _sys
    import types

    if "antenv.axon_hooks" in _sys.modules:
        return
    _sys.path.insert(0, "/root/.axon_site/trn_agent_boot")
    try:
        import trn_boot
    finally:
        _sys.path.pop(0)
    hook = trn_boot._ntff_profile_via_ctypes("/opt/axon/libaxon_pjrt.so")
    mod = types.ModuleType("antenv.axon_hooks")
    mod._hook = hook
    mod.get_axon_ntff_profile_hook = lambda: mod._hook
    mod.set_axon_ntff_profile_hook = lambda h: setattr(mod, "_hook", h)
    _sys.modules["antenv.axon_hooks"] = mod
    import antenv

    antenv.axon_hooks = mod


def _run(inputs, trace=False):
    from concourse.bass_utils import run_bass_kernel_spmd

    if trace:
        _install_ntff_shim()
    nc = _get_program()
    in_maps, feat = _host_prep(
        inputs["x"],
        inputs["conv1_w"],
        inputs["conv1_b"],
        inputs["conv2_w"],
        inputs["conv2_b"],
        inputs["reg_w"],
    )
    br = run_bass_kernel_spmd(nc, in_maps, core_ids=list(range(NCORES)), trace=trace)
    out = _host_post(br.results, feat, inputs["reg_w"], inputs["reg_b"])
    return out, br


def kernel(**inputs) -> np.ndarray:
    out, _ = _run(inputs, trace=False)
    return out
